# revision 1
# baseline (speedup 1.0000x reference)
"""ConvLSTM (B=4, T=8, C=HID=256, H=W=32, 3x3 SAME convs) on 8 TRN2 NeuronCores.

Sharding: data-parallel over batch (4) x spatial halves of H (2) = 8 cores,
zero inter-core communication. Each core computes its half's rows plus a
shrinking halo margin: at step t it computes 23-t rows; wrong values erode
inward from the un-owned edge at 1 row/step, leaving exactly the owned 16
rows correct after T=8 steps. Upper halves are row-flipped host-side (with
dy-flipped kernels) so all 8 cores run the same SPMD instruction stream.

Compute: conv as 36 PE matmuls per output tile (2 convs x 2 ic-tiles x 9
taps), float32r (fp32 rounded to 11-bit mantissa) at full PE rate, fp32
PSUM accumulation. Gates: sigmoid on ScalarE (bias fused), relu+bias on
VectorE. State update on VectorE. x-conv matmuls are issued before h-conv
matmuls in each chunk so the PE stays busy across the recurrence boundary.
"""
import numpy as np
from contextlib import ExitStack

import concourse.bass as bass
import concourse.tile as tile
from concourse import bacc, mybir
from concourse.bass_utils import run_bass_kernel_spmd

F32R = mybir.dt.float32r
F32 = mybir.dt.float32
AF = mybir.ActivationFunctionType
ALU = mybir.AluOpType

N_CORES = 8
T = 8
ROWS = 26          # h/x buffer rows: p=0 is the y=-1 zero row, p=1..24 = y=0..23
WC = 34            # padded width
PLANE = ROWS * WC  # 884
CROWS = 23         # c buffer rows (max computed rows), 23*32 = 736 per ic-tile
CPL = CROWS * 32

_cache = {}

# tap order: dy=1 row first so the start=True matmul is always full-width
# (dy=0 taps at the top chunk are shrunk by one row — they'd read the
# permanent zero row for output row 1, contributing nothing)
KORD = [3, 4, 5, 0, 1, 2, 6, 7, 8]


def _chunks(t):
    rt = 23 - t
    if rt > 16:
        r1 = (rt + 1) // 2
        return [(1, r1), (1 + r1, rt - r1)]
    return [(1, rt)]


def _build_nc():
    nc = bacc.Bacc("TRN2", target_bir_lowering=False, debug=False,
                   num_devices=N_CORES)
    x_d = nc.dram_tensor("xb", [T, 128, 2 * PLANE], F32R, kind="ExternalInput").ap()
    w_d = nc.dram_tensor("w", [36, 128, 1024], F32R, kind="ExternalInput").ap()
    b_d = nc.dram_tensor("bias", [128, 8], F32, kind="ExternalInput").ap()
    z_d = nc.dram_tensor("hz", [128, 2 * PLANE], F32R, kind="ExternalInput").ap()
    out_d = nc.dram_tensor("hout", [2, 128, 512], F32R, kind="ExternalOutput").ap()

    with tile.TileContext(nc) as tc, ExitStack() as ctx:
        wp = ctx.enter_context(tc.tile_pool(name="wp", bufs=1))
        xp = ctx.enter_context(tc.tile_pool(name="xp", bufs=2))
        hp = ctx.enter_context(tc.tile_pool(name="hp", bufs=1))
        cp = ctx.enter_context(tc.tile_pool(name="cp", bufs=1))
        bp = ctx.enter_context(tc.tile_pool(name="bp", bufs=1))
        gp = ctx.enter_context(tc.tile_pool(name="gp", bufs=10))
        tp = ctx.enter_context(tc.tile_pool(name="tp", bufs=3))
        pp = ctx.enter_context(tc.tile_pool(name="pp", bufs=8, space="PSUM"))

        bt = bp.tile([128, 8], F32, tag="bias")
        nc.sync.dma_start(bt[:], b_d[:])

        ha = hp.tile([128, 2 * PLANE], F32R, tag="ha")
        hb = hp.tile([128, 2 * PLANE], F32R, tag="hb")
        ct = cp.tile([128, 2 * CPL], F32, tag="c")
        nc.vector.memset(ct[:], 0.0)
        hbufs = [ha, hb]

        # x and the h zero-fills ride the gpsimd (SWDGE) queue so they never
        # wait behind the 18.9MB weight stream on the sync (HWDGE) queue.
        # memset can't emit float32r (ISA check) — zero-init h via DMA.
        x0 = xp.tile([128, 2 * PLANE], F32R, tag="x")
        for lo, hi in ((0, 544), (PLANE, PLANE + 544),
                       (544, PLANE), (PLANE + 544, 2 * PLANE)):
            nc.gpsimd.dma_start(x0[:, lo:hi], x_d[0][:, lo:hi])
        nc.gpsimd.dma_start(hb[:], z_d[:])
        nc.gpsimd.dma_start(ha[:], z_d[:])

        # One tile per weight slice so a matmul only waits on the slice it
        # reads. Gates are host-reordered to [i, o, g, f]: t=0 skips the f
        # octiles, so the x-weight slices' i/o/g columns load first and the
        # f columns + all h-weights follow.
        wxs = [wp.tile([128, 768], F32R, tag=f"wx{j}", name=f"wx{j}")
               for j in range(18)]
        wfs = [wp.tile([128, 256], F32R, tag=f"wf{j}", name=f"wf{j}")
               for j in range(18)]
        whs = [wp.tile([128, 1024], F32R, tag=f"wh{j}", name=f"wh{j}")
               for j in range(18)]
        for j in range(18):
            nc.sync.dma_start(wxs[j][:], w_d[j][:, :768])
        for j in range(18):
            nc.sync.dma_start(wfs[j][:], w_d[j][:, 768:])
        for j in range(18):
            nc.sync.dma_start(whs[j][:], w_d[18 + j])

        def wslice(j, o):
            if j < 18:
                if o < 6:
                    return wxs[j][:, o * 128:(o + 1) * 128]
                return wfs[j][:, (o - 6) * 128:(o - 5) * 128]
            return whs[j - 18][:, o * 128:(o + 1) * 128]

        for t in range(T):
            h_in, h_out = hbufs[t % 2], hbufs[(t + 1) % 2]
            if t == 0:
                xt = x0
            else:
                xt = xp.tile([128, 2 * PLANE], F32R, tag="x")
                nc.gpsimd.dma_start(xt[:], x_d[t])
            xv = xt[:].rearrange("p (i r c) -> p i r c", i=2, r=ROWS, c=WC)
            hv = h_in[:].rearrange("p (i r c) -> p i r c", i=2, r=ROWS, c=WC)
            hov = h_out[:].rearrange("p (i r c) -> p i r c", i=2, r=ROWS, c=WC)

            # t=0: h_0 == 0, so skip all h-conv matmuls; f-gate is unused
            # (f*c_0 == 0), so skip its two octiles entirely.
            # octile order (host-reordered): 0,1=i  2,3=o  4,5=g  6,7=f
            octs = [0, 1, 2, 3, 4, 5] if t == 0 else list(range(8))
            for (q, r) in _chunks(t):
                n = r * 32
                ps_tiles = {}
                # x-conv half first: independent of the recurrence, keeps the
                # PE busy while the previous step's state update drains.
                # At t=0 the weight slices are still streaming in from HBM,
                # so iterate j-major to consume them in arrival order.
                def emit_mm(ps, src, j, o, it, k, start, stop):
                    dy, dx = k // 3, k % 3
                    if q == 1 and dy == 0:
                        # top chunk: dy=0 tap of output row 1 reads the
                        # permanent zero row -> drop that row from the MM
                        nc.tensor.matmul(
                            ps[:, 32:], wslice(j, o),
                            src[:, it, 1: r, dx: dx + 32],
                            start=start, stop=stop, skip_group_check=True)
                    else:
                        nc.tensor.matmul(
                            ps[:], wslice(j, o),
                            src[:, it, q + dy - 1: q + dy - 1 + r,
                                dx: dx + 32],
                            start=start, stop=stop, skip_group_check=True)

                if t == 0:
                    for o in octs:
                        ps_tiles[o] = pp.tile([128, n], F32, tag="ps",
                                              name=f"ps{o}")
                    for it in range(2):
                        for k in KORD:
                            j = it * 9 + k
                            for o in octs:
                                emit_mm(ps_tiles[o], xv, j, o, it, k,
                                        start=(it == 0 and k == KORD[0]),
                                        stop=(it == 1 and k == KORD[-1]))
                else:
                    for o in octs:
                        ps = pp.tile([128, n], F32, tag="ps")
                        ps_tiles[o] = ps
                        for it in range(2):
                            for k in KORD:
                                emit_mm(ps, xv, it * 9 + k, o, it, k,
                                        start=(it == 0 and k == KORD[0]),
                                        stop=False)
                if t > 0:
                    for o in octs:
                        ps = ps_tiles[o]
                        for it in range(2):
                            for k in KORD:
                                emit_mm(ps, hv, 18 + it * 9 + k, o, it, k,
                                        start=False,
                                        stop=(it == 1 and k == KORD[-1]))
                gts = {}
                for o in octs:
                    gt = gp.tile([128, n], F32, tag="g")
                    gts[o] = gt
                    if o < 4 or o >= 6:  # i, o, f -> sigmoid; g -> relu
                        nc.scalar.activation(gt[:], ps_tiles[o][:], AF.Sigmoid,
                                             bias=bt[:, o:o + 1])
                    else:
                        nc.vector.tensor_scalar(gt[:], ps_tiles[o][:],
                                                bt[:, o:o + 1], 0.0,
                                                ALU.add, ALU.max)
                for hi in range(2):
                    gi, go, gg = gts[0 + hi], gts[2 + hi], gts[4 + hi]
                    c0 = hi * CPL + (q - 1) * 32
                    cs = ct[:, c0: c0 + n]
                    if t == 0:
                        nc.vector.tensor_mul(cs, gi[:], gg[:])
                    else:
                        gf = gts[6 + hi]
                        nc.vector.tensor_mul(gg[:], gi[:], gg[:])
                        nc.vector.tensor_mul(cs, gf[:], cs)
                        nc.vector.tensor_add(cs, cs, gg[:])
                    cr = tp.tile([128, n], F32, tag="cr")
                    nc.vector.tensor_scalar_max(cr[:], cs, 0.0)
                    nc.vector.tensor_mul(hov[:, hi, q: q + r, 1: 33], go[:], cr[:])

        hf = hbufs[T % 2][:].rearrange("p (i r c) -> p i r c", i=2, r=ROWS, c=WC)
        for it in range(2):
            nc.sync.dma_start(out_d[it], hf[:, it, 1: 17, 1: 33])

    nc.compile()
    return nc


def _round_f32r(a):
    b = np.ascontiguousarray(a, dtype=np.float32).view(np.uint32)
    b = (b + np.uint32(0x7FF) + ((b >> np.uint32(12)) & np.uint32(1))) \
        & np.uint32(0xFFFFF000)
    return b.view(np.float32)


GATE_PERM = [0, 2, 3, 1]  # reorder [i, f, o, g] -> [i, o, g, f]


def _prep_weights(wx, wh, flip):
    ws = np.stack([np.asarray(wx), np.asarray(wh)])  # [2, 1024, 256, 3, 3]
    if flip:
        ws = ws[:, :, :, ::-1, :]
    # [cv, gate, ht, ch, it, ic, dy, dx] -> [cv, it, dy, dx, ic, gate, ht, ch]
    ws = ws.reshape(2, 4, 2, 128, 2, 128, 3, 3)[:, GATE_PERM]
    ws = ws.transpose(0, 4, 6, 7, 5, 1, 2, 3)
    return _round_f32r(ws.reshape(36, 128, 1024))


def _prep_x(xb, flip):
    # xb: [T, 256, 32, 32] for one batch element -> [T, 128, 2*PLANE]
    xc = np.asarray(xb)
    if flip:
        xc = xc[:, :, ::-1, :]
    buf = np.zeros((T, 2, 128, ROWS, WC), dtype=np.float32)
    for it in range(2):
        buf[:, it, :, 1:25, 1:33] = xc[:, it * 128:(it + 1) * 128, 0:24, :]
    buf = buf.reshape(T, 2, 128, PLANE).transpose(0, 2, 1, 3)
    return _round_f32r(np.ascontiguousarray(buf).reshape(T, 128, 2 * PLANE))


def kernel(x, wx, wh, bh):
    x = np.asarray(x, dtype=np.float32)
    B = x.shape[0]
    bias = np.ascontiguousarray(
        np.asarray(bh, dtype=np.float32).reshape(4, 2, 128)[GATE_PERM]
        .transpose(2, 0, 1).reshape(128, 8))

    w_lo = _prep_weights(wx, wh, flip=False)
    w_hi = _prep_weights(wx, wh, flip=True)

    in_maps = []
    for c in range(N_CORES):
        b, half = c // 2, c % 2
        in_maps.append({
            "xb": _prep_x(x[b], flip=bool(half)),
            "w": w_hi if half else w_lo,
            "bias": bias,
            "hz": np.zeros((128, 2 * PLANE), dtype=np.float32),
        })

    if "nc" not in _cache:
        _cache["nc"] = _build_nc()
    nc = _cache["nc"]

    res = run_bass_kernel_spmd(nc, in_maps, core_ids=list(range(N_CORES)))
    _cache["last_results"] = res

    out = np.zeros((B, 256, 32, 32), dtype=np.float32)
    for c in range(N_CORES):
        b, half = c // 2, c % 2
        h = res.results[c]["hout"].reshape(2, 128, 16, 32)
        h = np.concatenate([h[0], h[1]], axis=0)  # [256, 16, 32]
        if half:
            out[b, :, 16:32, :] = h[:, ::-1, :]
        else:
            out[b, :, 0:16, :] = h
    return out



# revision 9
# speedup vs baseline: 1.1249x; 1.1249x over previous
"""ConvLSTM (B=4, T=8, C=HID=256, H=W=32, 3x3 SAME convs) on 8 TRN2 NeuronCores.

Sharding: data-parallel over batch (4) x spatial halves of H (2) = 8 cores,
zero inter-core communication. Each core computes its half's rows plus a
shrinking halo margin: at step t it computes 23-t rows; wrong values erode
inward from the un-owned edge at 1 row/step, leaving exactly the owned 16
rows correct after T=8 steps. Upper halves are row-flipped host-side (with
dy-flipped kernels) so all 8 cores run the same SPMD instruction stream.

Compute: conv as 36 PE matmuls per output tile (2 convs x 2 ic-tiles x 9
taps), bf16 operands (enables fast-weight-load so LDWEIGHTS hides behind
the matmul stream), fp32 PSUM accumulation. Gates: sigmoid on ScalarE
(bias fused), relu+bias on VectorE. State update on VectorE. x-conv
matmuls are issued before h-conv matmuls in each chunk so the PE stays
busy across the recurrence boundary.
"""
import numpy as np
import ml_dtypes
from contextlib import ExitStack

import concourse.bass as bass
import concourse.tile as tile
from concourse import bacc, mybir
from concourse.bass_utils import run_bass_kernel_spmd

BF16 = mybir.dt.bfloat16
NPBF = ml_dtypes.bfloat16
F32 = mybir.dt.float32
AF = mybir.ActivationFunctionType
ALU = mybir.AluOpType

N_CORES = 8
T = 8
ROWS = 26          # h/x buffer rows: p=0 is the y=-1 zero row, p=1..24 = y=0..23
WC = 34            # padded width
PLANE = ROWS * WC  # 884
CROWS = 23         # c buffer rows (max computed rows), 23*32 = 736 per ic-tile
CPL = CROWS * 32

_cache = {}

# tap order: dy=1 row first so the start=True matmul is always full-width
# (dy=0 taps at the top chunk are shrunk by one row — they'd read the
# permanent zero row for output row 1, contributing nothing)
KORD = [3, 4, 5, 0, 1, 2, 6, 7, 8]


def _chunks(t):
    rt = 23 - t
    if rt > 16:
        r1 = (rt + 1) // 2
        return [(1, r1), (1 + r1, rt - r1)]
    return [(1, rt)]


def _build_nc():
    nc = bacc.Bacc("TRN2", target_bir_lowering=False, debug=False,
                   num_devices=N_CORES)
    x_d = nc.dram_tensor("xb", [T, 128, 2 * PLANE], BF16, kind="ExternalInput").ap()
    w_d = nc.dram_tensor("w", [36, 128, 1024], BF16, kind="ExternalInput").ap()
    b_d = nc.dram_tensor("bias", [128, 8], F32, kind="ExternalInput").ap()
    z_d = nc.dram_tensor("hz", [128, 2 * PLANE], BF16, kind="ExternalInput").ap()
    out_d = nc.dram_tensor("hout", [2, 128, 512], BF16, kind="ExternalOutput").ap()

    with tile.TileContext(nc) as tc, ExitStack() as ctx:
        wp = ctx.enter_context(tc.tile_pool(name="wp", bufs=1))
        xp = ctx.enter_context(tc.tile_pool(name="xp", bufs=2))
        hp = ctx.enter_context(tc.tile_pool(name="hp", bufs=1))
        cp = ctx.enter_context(tc.tile_pool(name="cp", bufs=1))
        bp = ctx.enter_context(tc.tile_pool(name="bp", bufs=1))
        gp = ctx.enter_context(tc.tile_pool(name="gp", bufs=10))
        tp = ctx.enter_context(tc.tile_pool(name="tp", bufs=3))
        pp = ctx.enter_context(tc.tile_pool(name="pp", bufs=8, space="PSUM"))

        bt = bp.tile([128, 8], F32, tag="bias")
        nc.sync.dma_start(bt[:], b_d[:])

        ha = hp.tile([128, 2 * PLANE], BF16, tag="ha")
        hb = hp.tile([128, 2 * PLANE], BF16, tag="hb")
        ct = cp.tile([128, 2 * CPL], F32, tag="c")
        nc.vector.memset(ct[:], 0.0)
        hbufs = [ha, hb]

        # x and the h zero-fills ride the gpsimd (SWDGE) queue so they never
        # wait behind the 9.4MB weight stream on the sync (HWDGE) queue.
        x0 = xp.tile([128, 2 * PLANE], BF16, tag="x")
        for lo, hi in ((0, 544), (PLANE, PLANE + 544),
                       (544, PLANE), (PLANE + 544, 2 * PLANE)):
            nc.gpsimd.dma_start(x0[:, lo:hi], x_d[0][:, lo:hi])
        nc.gpsimd.dma_start(hb[:], z_d[:])
        nc.gpsimd.dma_start(ha[:], z_d[:])

        # One tile per weight slice so a matmul only waits on the slice it
        # reads. Gates are host-reordered to [i, o, g, f]: t=0 skips the f
        # octiles, so the x-weight slices' i/o/g columns load first and the
        # f columns + all h-weights follow. DMAs are issued in first-use
        # (KORD, it-major) order so the t=0 matmuls start sooner.
        wxs = [wp.tile([128, 768], BF16, tag=f"wx{j}", name=f"wx{j}")
               for j in range(18)]
        wfs = [wp.tile([128, 256], BF16, tag=f"wf{j}", name=f"wf{j}")
               for j in range(18)]
        whs = [wp.tile([128, 1024], BF16, tag=f"wh{j}", name=f"wh{j}")
               for j in range(18)]
        JORD = [it * 9 + k for it in range(2) for k in KORD]
        for j in JORD:
            nc.sync.dma_start(wxs[j][:], w_d[j][:, :768])
        for j in JORD:
            nc.sync.dma_start(wfs[j][:], w_d[j][:, 768:])
        for j in JORD:
            nc.sync.dma_start(whs[j][:], w_d[18 + j])

        def wslice(j, o):
            if j < 18:
                if o < 6:
                    return wxs[j][:, o * 128:(o + 1) * 128]
                return wfs[j][:, (o - 6) * 128:(o - 5) * 128]
            return whs[j - 18][:, o * 128:(o + 1) * 128]

        for t in range(T):
            h_in, h_out = hbufs[t % 2], hbufs[(t + 1) % 2]
            if t == 0:
                xt = x0
            else:
                xt = xp.tile([128, 2 * PLANE], BF16, tag="x")
                nc.gpsimd.dma_start(xt[:], x_d[t])
            xv = xt[:].rearrange("p (i r c) -> p i r c", i=2, r=ROWS, c=WC)
            hv = h_in[:].rearrange("p (i r c) -> p i r c", i=2, r=ROWS, c=WC)
            hov = h_out[:].rearrange("p (i r c) -> p i r c", i=2, r=ROWS, c=WC)

            # t=0: h_0 == 0, so skip all h-conv matmuls; f-gate is unused
            # (f*c_0 == 0), so skip its two octiles entirely.
            # octile order (host-reordered): 0,1=i  2,3=o  4,5=g  6,7=f
            octs = [0, 1, 2, 3, 4, 5] if t == 0 else list(range(8))
            for (q, r) in _chunks(t):
                n = r * 32
                ps_tiles = {}
                # x-conv half first: independent of the recurrence, keeps the
                # PE busy while the previous step's state update drains.
                # At t=0 the weight slices are still streaming in from HBM,
                # so iterate j-major to consume them in arrival order.
                def emit_mm(ps, src, j, o, it, k, start, stop):
                    dy, dx = k // 3, k % 3
                    if q == 1 and dy == 0:
                        # top chunk: dy=0 tap of output row 1 reads the
                        # permanent zero row -> drop that row from the MM
                        nc.tensor.matmul(
                            ps[:, 32:], wslice(j, o),
                            src[:, it, 1: r, dx: dx + 32],
                            start=start, stop=stop, skip_group_check=True)
                    else:
                        nc.tensor.matmul(
                            ps[:], wslice(j, o),
                            src[:, it, q + dy - 1: q + dy - 1 + r,
                                dx: dx + 32],
                            start=start, stop=stop, skip_group_check=True)

                if t == 0:
                    for o in octs:
                        ps_tiles[o] = pp.tile([128, n], F32, tag="ps",
                                              name=f"ps{o}")
                    for it in range(2):
                        for k in KORD:
                            j = it * 9 + k
                            for o in octs:
                                emit_mm(ps_tiles[o], xv, j, o, it, k,
                                        start=(it == 0 and k == KORD[0]),
                                        stop=(it == 1 and k == KORD[-1]))
                else:
                    for o in octs:
                        ps = pp.tile([128, n], F32, tag="ps")
                        ps_tiles[o] = ps
                        for it in range(2):
                            for k in KORD:
                                emit_mm(ps, xv, it * 9 + k, o, it, k,
                                        start=(it == 0 and k == KORD[0]),
                                        stop=False)
                if t > 0:
                    for o in octs:
                        ps = ps_tiles[o]
                        for it in range(2):
                            for k in KORD:
                                emit_mm(ps, hv, 18 + it * 9 + k, o, it, k,
                                        start=False,
                                        stop=(it == 1 and k == KORD[-1]))
                gts = {}
                for o in octs:
                    gt = gp.tile([128, n], F32, tag="g")
                    gts[o] = gt
                    if o < 4 or o >= 6:  # i, o, f -> sigmoid; g -> relu
                        nc.scalar.activation(gt[:], ps_tiles[o][:], AF.Sigmoid,
                                             bias=bt[:, o:o + 1])
                    else:
                        nc.vector.tensor_scalar(gt[:], ps_tiles[o][:],
                                                bt[:, o:o + 1], 0.0,
                                                ALU.add, ALU.max)
                for hi in range(2):
                    gi, go, gg = gts[0 + hi], gts[2 + hi], gts[4 + hi]
                    c0 = hi * CPL + (q - 1) * 32
                    cs = ct[:, c0: c0 + n]
                    if t == 0:
                        nc.vector.tensor_mul(cs, gi[:], gg[:])
                    else:
                        gf = gts[6 + hi]
                        nc.vector.tensor_mul(gg[:], gi[:], gg[:])
                        nc.vector.tensor_mul(cs, gf[:], cs)
                        nc.vector.tensor_add(cs, cs, gg[:])
                    cr = tp.tile([128, n], F32, tag="cr")
                    nc.vector.tensor_scalar_max(cr[:], cs, 0.0)
                    nc.vector.tensor_mul(hov[:, hi, q: q + r, 1: 33], go[:], cr[:])

        hf = hbufs[T % 2][:].rearrange("p (i r c) -> p i r c", i=2, r=ROWS, c=WC)
        for it in range(2):
            nc.sync.dma_start(out_d[it], hf[:, it, 1: 17, 1: 33])

    nc.compile()
    return nc


GATE_PERM = [0, 2, 3, 1]  # reorder [i, f, o, g] -> [i, o, g, f]


def _prep_weights(wx, wh, flip):
    ws = np.stack([np.asarray(wx), np.asarray(wh)])  # [2, 1024, 256, 3, 3]
    if flip:
        ws = ws[:, :, :, ::-1, :]
    # [cv, gate, ht, ch, it, ic, dy, dx] -> [cv, it, dy, dx, ic, gate, ht, ch]
    ws = ws.reshape(2, 4, 2, 128, 2, 128, 3, 3)[:, GATE_PERM]
    ws = ws.transpose(0, 4, 6, 7, 5, 1, 2, 3)
    return np.ascontiguousarray(ws.reshape(36, 128, 1024)).astype(NPBF)


def _prep_x(xb, flip):
    # xb: [T, 256, 32, 32] for one batch element -> [T, 128, 2*PLANE]
    xc = np.asarray(xb)
    if flip:
        xc = xc[:, :, ::-1, :]
    buf = np.zeros((T, 2, 128, ROWS, WC), dtype=np.float32)
    for it in range(2):
        buf[:, it, :, 1:25, 1:33] = xc[:, it * 128:(it + 1) * 128, 0:24, :]
    buf = buf.reshape(T, 2, 128, PLANE).transpose(0, 2, 1, 3)
    return np.ascontiguousarray(buf).reshape(T, 128, 2 * PLANE).astype(NPBF)


def kernel(x, wx, wh, bh):
    x = np.asarray(x, dtype=np.float32)
    B = x.shape[0]
    bias = np.ascontiguousarray(
        np.asarray(bh, dtype=np.float32).reshape(4, 2, 128)[GATE_PERM]
        .transpose(2, 0, 1).reshape(128, 8))

    w_lo = _prep_weights(wx, wh, flip=False)
    w_hi = _prep_weights(wx, wh, flip=True)

    in_maps = []
    for c in range(N_CORES):
        b, half = c // 2, c % 2
        in_maps.append({
            "xb": _prep_x(x[b], flip=bool(half)),
            "w": w_hi if half else w_lo,
            "bias": bias,
            "hz": np.zeros((128, 2 * PLANE), dtype=NPBF),
        })

    if "nc" not in _cache:
        _cache["nc"] = _build_nc()
    nc = _cache["nc"]

    res = run_bass_kernel_spmd(nc, in_maps, core_ids=list(range(N_CORES)))
    _cache["last_results"] = res

    out = np.zeros((B, 256, 32, 32), dtype=np.float32)
    for c in range(N_CORES):
        b, half = c // 2, c % 2
        h = res.results[c]["hout"].astype(np.float32).reshape(2, 128, 16, 32)
        h = np.concatenate([h[0], h[1]], axis=0)  # [256, 16, 32]
        if half:
            out[b, :, 16:32, :] = h[:, ::-1, :]
        else:
            out[b, :, 0:16, :] = h
    return out



# revision 14
# speedup vs baseline: 1.2475x; 1.1089x over previous
"""ConvLSTM (B=4, T=8, C=HID=256, H=W=32, 3x3 SAME convs) on 8 TRN2 NeuronCores.

Sharding: data-parallel over batch (4) x spatial halves of H (2) = 8 cores.
Each core owns 16 rows and computes exactly 16 rows per step; the single
boundary row of h it needs from its partner is exchanged each step with a
2-core CC AllReduce (sum of both boundary rows staged in DRAM, partner row
recovered by subtracting our own contribution — keeps the instruction
stream SPMD-identical). Upper halves are row-flipped host-side (with
dy-flipped kernels) so all 8 cores run the same program.

Compute: conv as 36 PE matmuls per step per octile (2 convs x 2 ic-tiles x
9 taps) at bf16 (fast-weight-load keeps LDWEIGHTS hidden), fp32 PSUM
accumulation. Gates: sigmoid on ScalarE (bias fused), relu+bias on
VectorE. State update on VectorE. x-conv matmuls are issued before h-conv
matmuls so the PE stays busy while the halo row is in flight.
"""
import numpy as np
import ml_dtypes
from contextlib import ExitStack

import concourse.bass as bass
import concourse.tile as tile
from concourse import bacc, mybir
from concourse.bass_utils import run_bass_kernel_spmd

BF16 = mybir.dt.bfloat16
NPBF = ml_dtypes.bfloat16
F32 = mybir.dt.float32
AF = mybir.ActivationFunctionType
ALU = mybir.AluOpType

N_CORES = 8
T = 8
ROWS = 18          # p=0 zero row, p=1..16 owned rows 0..15, p=17 halo row 16
WC = 34            # padded width
PLANE = ROWS * WC  # 612
CROWS = 16         # c rows (owned only), 16*32 = 512 per ic-tile
CPL = CROWS * 32
R = 16             # rows computed per step
N = R * 32         # 512: matmul free dim / psum bank width

PAIRS = [[0, 1], [2, 3], [4, 5], [6, 7]]

_cache = {}

# tap order: dy=1 row first so the start=True matmul is always full-width
# (the dy=0 tap of output row 1 reads the permanent zero row — dropped)
KORD = [3, 4, 5, 0, 1, 2, 6, 7, 8]


def _build_nc():
    nc = bacc.Bacc("TRN2", target_bir_lowering=False, debug=False,
                   num_devices=N_CORES)
    x_d = nc.dram_tensor("xb", [T, 128, 2 * PLANE], BF16, kind="ExternalInput").ap()
    w_d = nc.dram_tensor("w", [36, 128, 1024], BF16, kind="ExternalInput").ap()
    b_d = nc.dram_tensor("bias", [128, 8], F32, kind="ExternalInput").ap()
    z_d = nc.dram_tensor("hz", [128, 2 * PLANE], BF16, kind="ExternalInput").ap()
    out_d = nc.dram_tensor("hout", [2, 128, 512], BF16, kind="ExternalOutput").ap()
    ccb = nc.dram_tensor("ccb", [128, 64], F32).ap()  # boundary-row mailbox

    with tile.TileContext(nc) as tc, ExitStack() as ctx:
        wp = ctx.enter_context(tc.tile_pool(name="wp", bufs=1))
        xp = ctx.enter_context(tc.tile_pool(name="xp", bufs=2))
        hp = ctx.enter_context(tc.tile_pool(name="hp", bufs=1))
        cp = ctx.enter_context(tc.tile_pool(name="cp", bufs=1))
        bp = ctx.enter_context(tc.tile_pool(name="bp", bufs=1))
        gp = ctx.enter_context(tc.tile_pool(name="gp", bufs=10))
        tp = ctx.enter_context(tc.tile_pool(name="tp", bufs=3))
        ep = ctx.enter_context(tc.tile_pool(name="ep", bufs=4))
        pp = ctx.enter_context(tc.tile_pool(name="pp", bufs=8, space="PSUM"))

        bt = bp.tile([128, 8], F32, tag="bias")
        nc.sync.dma_start(bt[:], b_d[:])

        ha = hp.tile([128, 2 * PLANE], BF16, tag="ha")
        hb = hp.tile([128, 2 * PLANE], BF16, tag="hb")
        ct = cp.tile([128, 2 * CPL], F32, tag="c")
        nc.vector.memset(ct[:], 0.0)
        hbufs = [ha, hb]

        # x and the h zero-fills ride the gpsimd (SWDGE) queue so they never
        # wait behind the 9.4MB weight stream on the sync (HWDGE) queue.
        x0 = xp.tile([128, 2 * PLANE], BF16, tag="x")
        nc.gpsimd.dma_start(x0[:, :PLANE], x_d[0][:, :PLANE])
        nc.gpsimd.dma_start(x0[:, PLANE:], x_d[0][:, PLANE:])
        nc.gpsimd.dma_start(hb[:], z_d[:])
        nc.gpsimd.dma_start(ha[:], z_d[:])

        # One tile per weight slice so a matmul only waits on the slice it
        # reads. Gates are host-reordered to [i, o, g, f]: t=0 skips the f
        # octiles, so the x-weight slices' i/o/g columns load first and the
        # f columns + all h-weights follow. DMAs are issued in first-use
        # (KORD, it-major) order so the t=0 matmuls start sooner.
        wxs = [wp.tile([128, 768], BF16, tag=f"wx{j}", name=f"wx{j}")
               for j in range(18)]
        wfs = [wp.tile([128, 256], BF16, tag=f"wf{j}", name=f"wf{j}")
               for j in range(18)]
        whs = [wp.tile([128, 1024], BF16, tag=f"wh{j}", name=f"wh{j}")
               for j in range(18)]
        JORD = [it * 9 + k for it in range(2) for k in KORD]
        for j in JORD:
            nc.sync.dma_start(wxs[j][:], w_d[j][:, :768])
        for j in JORD:
            nc.sync.dma_start(wfs[j][:], w_d[j][:, 768:])
        for j in JORD:
            nc.sync.dma_start(whs[j][:], w_d[18 + j])

        def wslice(j, o):
            if j < 18:
                if o < 6:
                    return wxs[j][:, o * 128:(o + 1) * 128]
                return wfs[j][:, (o - 6) * 128:(o - 5) * 128]
            return whs[j - 18][:, o * 128:(o + 1) * 128]

        next_x = x0
        for t in range(T):
            h_in, h_out = hbufs[t % 2], hbufs[(t + 1) % 2]
            xt = next_x
            if t < T - 1:
                # prefetch x for t+1 now, BEFORE the exchange ops put a
                # cc_sem wait on the gpsimd queue
                next_x = xp.tile([128, 2 * PLANE], BF16, tag="x")
                nc.gpsimd.dma_start(next_x[:], x_d[t + 1])
            xv = xt[:].rearrange("p (i r c) -> p i r c", i=2, r=ROWS, c=WC)
            hv = h_in[:].rearrange("p (i r c) -> p i r c", i=2, r=ROWS, c=WC)
            hov = h_out[:].rearrange("p (i r c) -> p i r c", i=2, r=ROWS, c=WC)

            # t=0: h_0 == 0, so skip all h-conv matmuls; f-gate is unused
            # (f*c_0 == 0), so skip its two octiles entirely.
            # octile order (host-reordered): 0,1=i  2,3=o  4,5=g  6,7=f
            octs = [0, 1, 2, 3, 4, 5] if t == 0 else list(range(8))

            def emit_mm(ps, src, j, o, it, k, start, stop):
                dy, dx = k // 3, k % 3
                if dy == 0:
                    # dy=0 tap of output row 1 reads the permanent zero
                    # row -> drop that row from the MM
                    nc.tensor.matmul(
                        ps[:, 32:], wslice(j, o),
                        src[:, it, 1: R, dx: dx + 32],
                        start=start, stop=stop, skip_group_check=True)
                else:
                    nc.tensor.matmul(
                        ps[:], wslice(j, o),
                        src[:, it, dy: dy + R, dx: dx + 32],
                        start=start, stop=stop, skip_group_check=True)

            ps_tiles = {}
            # x-conv half first: independent of the recurrence and of the
            # halo row in flight.  At t=0 the weight slices are still
            # streaming in from HBM, so iterate j-major to consume them in
            # arrival order.
            if t == 0:
                for o in octs:
                    ps_tiles[o] = pp.tile([128, N], F32, tag="ps",
                                          name=f"ps{o}")
                for it in range(2):
                    for k in KORD:
                        j = it * 9 + k
                        for o in octs:
                            emit_mm(ps_tiles[o], xv, j, o, it, k,
                                    start=(it == 0 and k == KORD[0]),
                                    stop=(it == 1 and k == KORD[-1]))
            else:
                for o in octs:
                    ps = pp.tile([128, N], F32, tag="ps")
                    ps_tiles[o] = ps
                    for it in range(2):
                        for k in KORD:
                            emit_mm(ps, xv, it * 9 + k, o, it, k,
                                    start=(it == 0 and k == KORD[0]),
                                    stop=False)
                for o in octs:
                    ps = ps_tiles[o]
                    for it in range(2):
                        for k in KORD:
                            emit_mm(ps, hv, 18 + it * 9 + k, o, it, k,
                                    start=False,
                                    stop=(it == 1 and k == KORD[-1]))
            gts = {}
            for o in octs:
                gt = gp.tile([128, N], F32, tag="g")
                gts[o] = gt
                if o < 4 or o >= 6:  # i, o, f -> sigmoid; g -> relu
                    nc.scalar.activation(gt[:], ps_tiles[o][:], AF.Sigmoid,
                                         bias=bt[:, o:o + 1])
                else:
                    nc.vector.tensor_scalar(gt[:], ps_tiles[o][:],
                                            bt[:, o:o + 1], 0.0,
                                            ALU.add, ALU.max)
            if t < T - 1:
                stage = ep.tile([128, 64], F32, tag="st", name="stage")
            else:
                stage = None
            for hi in range(2):
                gi, go, gg = gts[0 + hi], gts[2 + hi], gts[4 + hi]
                cs = ct[:, hi * CPL: hi * CPL + N]
                if t == 0:
                    nc.vector.tensor_mul(cs, gi[:], gg[:])
                else:
                    gf = gts[6 + hi]
                    nc.vector.tensor_mul(gg[:], gi[:], gg[:])
                    nc.vector.tensor_mul(cs, gf[:], cs)
                    nc.vector.tensor_add(cs, cs, gg[:])
                cr = tp.tile([128, N], F32, tag="cr")
                nc.vector.tensor_scalar_max(cr[:], cs, 0.0)
                nc.vector.tensor_mul(hov[:, hi, 1: 1 + R, 1: 33], go[:], cr[:])
                if t < T - 1:
                    # boundary row (local row 15) in fp32 for the exchange
                    nc.vector.tensor_mul(stage[:, hi * 32:(hi + 1) * 32],
                                         go[:, 480:512], cr[:, 480:512])
                else:
                    nc.sync.dma_start(out_d[hi], hov[:, hi, 1: 17, 1: 33])

            if t < T - 1:
                # pair-wise exchange of the boundary row: AllReduce(add) of
                # both cores' rows, partner row = sum - ours. The tile
                # framework tracks the DRAM mailbox, so it orders
                # DMA-out -> AllReduce -> DMA-in (and the WAR on reuse).
                nc.sync.dma_start(ccb[:], stage[:])
                nc.gpsimd.collective_compute(
                    "AllReduce", ALU.add, replica_groups=PAIRS,
                    ins=[ccb[:].opt()], outs=[ccb[:].opt()],
                )
                sumt = ep.tile([128, 64], F32, tag="sm")
                nc.gpsimd.dma_start(sumt[:], ccb[:])
                for hi in range(2):
                    nc.vector.tensor_sub(hov[:, hi, 17: 18, 1: 33],
                                         sumt[:, hi * 32:(hi + 1) * 32],
                                         stage[:, hi * 32:(hi + 1) * 32])

    nc.compile()
    return nc


GATE_PERM = [0, 2, 3, 1]  # reorder [i, f, o, g] -> [i, o, g, f]


def _prep_weights(wx, wh, flip):
    ws = np.stack([np.asarray(wx), np.asarray(wh)])  # [2, 1024, 256, 3, 3]
    if flip:
        ws = ws[:, :, :, ::-1, :]
    # [cv, gate, ht, ch, it, ic, dy, dx] -> [cv, it, dy, dx, ic, gate, ht, ch]
    ws = ws.reshape(2, 4, 2, 128, 2, 128, 3, 3)[:, GATE_PERM]
    ws = ws.transpose(0, 4, 6, 7, 5, 1, 2, 3)
    return np.ascontiguousarray(ws.reshape(36, 128, 1024)).astype(NPBF)


def _prep_x(xb, flip):
    # xb: [T, 256, 32, 32] for one batch element -> [T, 128, 2*PLANE]
    # rows 0..16 of the (possibly flipped) image: 16 owned + 1 halo row.
    xc = np.asarray(xb)
    if flip:
        xc = xc[:, :, ::-1, :]
    buf = np.zeros((T, 2, 128, ROWS, WC), dtype=np.float32)
    for it in range(2):
        buf[:, it, :, 1:18, 1:33] = xc[:, it * 128:(it + 1) * 128, 0:17, :]
    buf = buf.reshape(T, 2, 128, PLANE).transpose(0, 2, 1, 3)
    return np.ascontiguousarray(buf).reshape(T, 128, 2 * PLANE).astype(NPBF)


def kernel(x, wx, wh, bh):
    x = np.asarray(x, dtype=np.float32)
    B = x.shape[0]
    bias = np.ascontiguousarray(
        np.asarray(bh, dtype=np.float32).reshape(4, 2, 128)[GATE_PERM]
        .transpose(2, 0, 1).reshape(128, 8))

    w_lo = _prep_weights(wx, wh, flip=False)
    w_hi = _prep_weights(wx, wh, flip=True)

    in_maps = []
    for c in range(N_CORES):
        b, half = c // 2, c % 2
        in_maps.append({
            "xb": _prep_x(x[b], flip=bool(half)),
            "w": w_hi if half else w_lo,
            "bias": bias,
            "hz": np.zeros((128, 2 * PLANE), dtype=NPBF),
        })

    if "nc" not in _cache:
        _cache["nc"] = _build_nc()
    nc = _cache["nc"]

    res = run_bass_kernel_spmd(nc, in_maps, core_ids=list(range(N_CORES)))
    _cache["last_results"] = res

    out = np.zeros((B, 256, 32, 32), dtype=np.float32)
    for c in range(N_CORES):
        b, half = c // 2, c % 2
        h = res.results[c]["hout"].astype(np.float32).reshape(2, 128, 16, 32)
        h = np.concatenate([h[0], h[1]], axis=0)  # [256, 16, 32]
        if half:
            out[b, :, 16:32, :] = h[:, ::-1, :]
        else:
            out[b, :, 0:16, :] = h
    return out
